# revision 25
# baseline (speedup 1.0000x reference)
"""Trainium2 Bass kernel for CLIP attention pooling.

Reference computation (N=4096, D=1024, fp32):
    q = x @ Wq.T + bq
    k = x @ Wk.T + bk
    attn = softmax(q @ k.T, axis=-1)
    out = attn @ x

Math notes used here:
  * scores = q @ k.T = q @ (x Wk.T + bk).T = q @ Wk @ x.T + (q.bk) 1^T.
    The (q.bk) term is constant along the softmax axis, so softmax is
    invariant to it: bk never needs to be computed.
  * q @ Wk = x @ (Wq.T @ Wk) + bq @ Wk: the two projections fold into
    one matrix M = Wq.T @ Wk and a row c = bq @ Wk, both precomputed on
    the host (input-independent weight folding).
  * Therefore per core (512 query rows each):
        tT = M^T . xs^T + c          [D, 512]   (transposed layout)
        S  = t . x^T                 [512, 4096]
        P  = softmax(S)  (row-wise, two-pass with exact max)
        out = P @ x                  [512, 1024]
    This skips the full k projection (x @ Wk.T for all 4096 rows) on
    every core and roughly halves the FLOPs vs the naive row-parallel
    plan.

Implementation:
  * matmuls run as fp32r (TF32-like, ~11 mantissa bits, full PE rate at
    moving-dim >= 256) with fp32 PSUM accumulation.
  * the c row enters through an extra K=1 matmul (c x ones) in the tT
    accumulation groups - no vector-engine bias pass.
  * phase A runs contraction(e)-outer over 8 PSUM banks with per-chunk
    DMAs, so the first matmul only waits for one 128-row chunk of M/xs.
  * softmax: per-512-chunk partial maxes are reduced straight out of
    PSUM; exp runs on the scalar engine in 512-wide chunks (bias=-max,
    accum_out accumulating partial row sums), E in bf16.
  * P @ x: E tiles are PE-transposed (bf16) inside the output jt-loop,
    interleaved with the output matmuls (4 PSUM accumulator banks per
    pass, two passes over the 1024 output columns); 1/Z is applied on
    the PSUM->SBUF copy.
"""

import os
from contextlib import ExitStack

import numpy as np
import ml_dtypes

import concourse.bass as bass
import concourse.mybir as mybir
import concourse.tile as tile
from concourse import bacc
from concourse.bass_utils import run_bass_kernel_spmd
from concourse.masks import make_identity

N, D = 4096, 1024
NCORES = 8
R = N // NCORES  # 512 query rows per core
PT = 128  # partition tile
EC = D // PT  # 8 contraction chunks of the model dim
IT = R // PT  # 4 query tiles per core
JC = N // 512  # 8 key chunks of 512
JT = N // PT  # 32 key tiles of 128

F32 = mybir.dt.float32
F32R = mybir.dt.float32r
BF16 = mybir.dt.bfloat16
AX = mybir.AxisListType
AF = mybir.ActivationFunctionType


def _emit(nc: bass.Bass, tc: tile.TileContext, aps: dict):
    xTb, xTs, mw, cw, ones, xb, out = (
        aps["xTb"], aps["xTs"], aps["mw"], aps["cw"],
        aps["ones"], aps["xb"], aps["out"],
    )

    with ExitStack() as big:
        persist = big.enter_context(tc.tile_pool(name="persist", bufs=1))

        ident = persist.tile([PT, PT], BF16)
        make_identity(nc, ident)
        c_sb = persist.tile([1, D], F32R)
        ones_sb = persist.tile([1, R], F32R)

        tT_sb = persist.tile([PT, EC, R], F32R)
        # chunk 0 of the phase-B xT stream lives outside the weight pool's
        # address range so its DMA can run during phase A instead of waiting
        # for the weights to be released.
        xtj0 = persist.tile([PT, EC, 512], F32R)

        # ---- Phase A: tT = M^T.xs^T + c  (transposed layout)
        # e-outer over 8 PSUM banks; per-chunk DMAs so matmuls start after
        # the first chunk lands.
        with ExitStack() as pha:
            wpool = pha.enter_context(tc.tile_pool(name="wpool", bufs=1))
            apsum = pha.enter_context(tc.tile_pool(name="apsum", bufs=1, space="PSUM"))

            m_sb = wpool.tile([PT, EC, D], F32R)
            xts_sb = wpool.tile([PT, EC, R], F32R)

            m_r = mw.rearrange("(t p) d -> p t d", p=PT)
            xTs_r = xTs.rearrange("(t p) i -> p t i", p=PT)
            nc.sync.dma_start(m_sb[:, 0, 0:PT], m_r[:, 0, 0:PT])
            nc.sync.dma_start(xts_sb[:, 0, :], xTs_r[:, 0, :])
            nc.sync.dma_start(m_sb[:, 0, PT:D], m_r[:, 0, PT:D])
            for e in range(1, EC):
                nc.sync.dma_start(xts_sb[:, e, :], xTs_r[:, e, :])
                nc.sync.dma_start(m_sb[:, e, :], m_r[:, e, :])
            nc.sync.dma_start(c_sb, cw)
            nc.sync.dma_start(ones_sb, ones)
            nc.sync.dma_start(xtj0, xTb[0])

            tps = [
                apsum.tile([PT, R], F32, tag=f"tp{d}", name=f"tp{d}")
                for d in range(EC)
            ]
            for e in range(EC):
                for d in range(EC):
                    nc.tensor.matmul(
                        tps[d],
                        m_sb[:, e, d * PT : (d + 1) * PT],
                        xts_sb[:, e, :],
                        start=(e == 0),
                        stop=False,
                    )
            for d in range(EC):
                # bias row: tT[d_block, :] += c[d_block] (x) ones
                nc.tensor.matmul(
                    tps[d],
                    c_sb[:, d * PT : (d + 1) * PT],
                    ones_sb,
                    start=False,
                    stop=True,
                )
                if d % 2 == 0:
                    nc.vector.tensor_copy(tT_sb[:, d, :], tps[d])
                else:
                    nc.scalar.activation(tT_sb[:, d, :], tps[d], func=AF.Copy)

        # Pools for softmax state open after the weight pool closes so the
        # addresses can be reused.
        spool = big.enter_context(tc.tile_pool(name="spool", bufs=1))
        S_sb = [spool.tile([PT, N], F32, tag=f"S{i}", name=f"S{i}") for i in range(IT)]
        mxp = [spool.tile([PT, JC], F32, tag=f"mxp{i}", name=f"mxp{i}") for i in range(IT)]
        negmax = [spool.tile([PT, 1], F32, tag=f"nm{i}", name=f"nm{i}") for i in range(IT)]
        zpart = [spool.tile([PT, JC + 2], F32, tag=f"zp{i}", name=f"zp{i}") for i in range(IT)]
        zsum = [spool.tile([PT, 1], F32, tag=f"z{i}", name=f"z{i}") for i in range(IT)]
        rz = [spool.tile([PT, 1], F32, tag=f"rz{i}", name=f"rz{i}") for i in range(IT)]
        epool = big.enter_context(tc.tile_pool(name="epool", bufs=4))
        E_bf = [epool.tile([PT, N], BF16, tag="E", name=f"E{i}") for i in range(IT)]

        # ---- Phase B: S = t . x^T, chunked over j; partial maxes from PSUM
        with ExitStack() as phb:
            xtpool = phb.enter_context(tc.tile_pool(name="xtpool", bufs=3))
            spsum = phb.enter_context(tc.tile_pool(name="spsum", bufs=5, space="PSUM"))
            for j in range(JC):
                if j == 0:
                    xtj = xtj0
                else:
                    xtj = xtpool.tile([PT, EC, 512], F32R, tag="xtj", name="xtj")
                    nc.sync.dma_start(xtj, xTb[j])
                last_ps = []
                for i in range(IT):
                    ps = spsum.tile([PT, 512], F32, tag="Sp", name="Sp")
                    for d in range(EC):
                        nc.tensor.matmul(
                            ps,
                            tT_sb[:, d, i * PT : (i + 1) * PT],
                            xtj[:, d, :],
                            start=(d == 0),
                            stop=(d == EC - 1),
                        )
                    nc.vector.reduce_max(
                        out=mxp[i][:, j : j + 1], in_=ps, axis=AX.X
                    )
                    if j < JC - 1:
                        nc.vector.tensor_copy(
                            S_sb[i][:, j * 512 : (j + 1) * 512], ps
                        )
                    else:
                        # last chunk: maxes were emitted first; split the S
                        # copies across DVE and ACT so -max (and the exp
                        # chain behind it) clears the vector queue sooner.
                        last_ps.append(ps)
                for i, ps in enumerate(last_ps):
                    dst = S_sb[i][:, (JC - 1) * 512 : JC * 512]
                    if i % 2 == 0:
                        nc.vector.tensor_copy(dst, ps)
                    else:
                        nc.scalar.activation(dst, ps, func=AF.Copy)

        # ---- Phase B2: softmax. Chunked exp so the PE can resume quickly.
        for i in range(IT):
            nc.vector.reduce_max(out=negmax[i], in_=mxp[i], axis=AX.X, negate=True)
        for i in range(IT):
            # narrow first piece: unblocks the first E transposes early
            nc.scalar.activation(
                out=E_bf[i][:, 0:256],
                in_=S_sb[i][:, 0:256],
                func=AF.Exp,
                bias=negmax[i],
                scale=1.0,
                accum_out=zpart[i][:, JC : JC + 1],
            )
        for i in range(IT):
            nc.scalar.activation(
                out=E_bf[i][:, 256:512],
                in_=S_sb[i][:, 256:512],
                func=AF.Exp,
                bias=negmax[i],
                scale=1.0,
                accum_out=zpart[i][:, JC + 1 : JC + 2],
            )
        for j in range(1, JC):
            for i in range(IT):
                nc.scalar.activation(
                    out=E_bf[i][:, j * 512 : (j + 1) * 512],
                    in_=S_sb[i][:, j * 512 : (j + 1) * 512],
                    func=AF.Exp,
                    bias=negmax[i],
                    scale=1.0,
                    accum_out=zpart[i][:, j : j + 1],
                )
        for i in range(IT):
            nc.vector.reduce_sum(
                out=zsum[i], in_=zpart[i][:, 1 : JC + 2], axis=AX.X
            )
            nc.vector.reciprocal(rz[i], zsum[i])

        # ---- Phase T+C fused: out = P @ x. Two passes over i-halves; each
        # pass interleaves the E transposes for its two i-tiles with the
        # output matmuls (keeps the PE activity monitor warm) and accumulates
        # into 4 PSUM banks. 1/Z fused on the copy-out; pass-0 results are
        # copied out while pass 1 runs.
        etpool = big.enter_context(tc.tile_pool(name="etpool", bufs=1))
        ET_sb = etpool.tile([PT, JT, R], BF16)
        ocopy = big.enter_context(tc.tile_pool(name="ocopy", bufs=4))
        tpsum = big.enter_context(
            tc.tile_pool(name="tpsum", bufs=2, space="PSUM")
        )
        for h in range(2):
            with ExitStack() as phc:
                xbpool = phc.enter_context(
                    tc.tile_pool(name=f"xbpool{h}", bufs=6)
                )
                opsum = phc.enter_context(
                    tc.tile_pool(name=f"opsum{h}", bufs=1, space="PSUM")
                )
                ii = (2 * h, 2 * h + 1)
                xbjs = {}
                oacc = {
                    (i, dn): opsum.tile(
                        [PT, 512], F32, tag=f"o{i}_{dn}", name=f"o{i}_{dn}"
                    )
                    for i in ii
                    for dn in range(2)
                }
                LOOK = 2
                for jtv in range(JT - 1 + LOOK):
                    if jtv < JT - 1:
                        jt = jtv
                        pst = tpsum.tile([PT, 2 * PT], BF16, tag="tp", name="pst")
                        for k, i in enumerate(ii):
                            nc.tensor.transpose(
                                pst[:, k * PT : (k + 1) * PT],
                                E_bf[i][:, jt * PT : (jt + 1) * PT],
                                ident,
                            )
                        nc.vector.tensor_copy(
                            ET_sb[:, jt, h * 256 : (h + 1) * 256], pst
                        )
                        xbj = xbpool.tile([PT, D], BF16, tag="xbj", name="xbj")
                        nc.sync.dma_start(xbj, xb[jt * PT : (jt + 1) * PT, :])
                        xbjs[jt % 8] = xbj
                    if jtv >= LOOK:
                        jt = jtv - LOOK
                        for i in ii:
                            for dn in range(2):
                                nc.tensor.matmul(
                                    oacc[(i, dn)],
                                    ET_sb[:, jt, i * PT : (i + 1) * PT],
                                    xbjs[jt % 8][:, dn * 512 : (dn + 1) * 512],
                                    start=(jt == 0),
                                    stop=(jt == JT - 1),
                                )
                # last jt: transposes, then per-bank stop-matmul immediately
                # followed by its copy-out so copies overlap the other banks'
                # matmuls (and, for pass 0, the start of pass 1).
                jt = JT - 1
                pst = tpsum.tile([PT, 2 * PT], BF16, tag="tp", name="pst")
                for k, i in enumerate(ii):
                    nc.tensor.transpose(
                        pst[:, k * PT : (k + 1) * PT],
                        E_bf[i][:, jt * PT : (jt + 1) * PT],
                        ident,
                    )
                nc.vector.tensor_copy(
                    ET_sb[:, jt, h * 256 : (h + 1) * 256], pst
                )
                xbj = xbpool.tile([PT, D], BF16, tag="xbj", name="xbj")
                nc.sync.dma_start(xbj, xb[jt * PT : (jt + 1) * PT, :])
                xbjs[jt % 8] = xbj
                for i in ii:
                    for dn in range(2):
                        nc.tensor.matmul(
                            oacc[(i, dn)],
                            ET_sb[:, jt, i * PT : (i + 1) * PT],
                            xbjs[jt % 8][:, dn * 512 : (dn + 1) * 512],
                            start=False,
                            stop=True,
                        )
                        ot = ocopy.tile([PT, 512], F32, tag="ot", name="ot")
                        if dn == 0:
                            nc.vector.tensor_scalar_mul(ot, oacc[(i, dn)], rz[i])
                        else:
                            nc.scalar.activation(
                                ot, oacc[(i, dn)], func=AF.Copy, scale=rz[i]
                            )
                        nc.sync.dma_start(
                            out[i * PT : (i + 1) * PT, dn * 512 : (dn + 1) * 512],
                            ot,
                        )


def build():
    nc = bacc.Bacc(
        "TRN2",
        target_bir_lowering=False,
        debug=False,
        enable_asserts=False,
        num_devices=NCORES,
    )
    aps = {
        "xTb": nc.dram_tensor("xTb", [JC, PT, EC, 512], F32R, kind="ExternalInput").ap(),
        "xTs": nc.dram_tensor("xTs", [D, R], F32R, kind="ExternalInput").ap(),
        "mw": nc.dram_tensor("mw", [D, D], F32R, kind="ExternalInput").ap(),
        "cw": nc.dram_tensor("cw", [1, D], F32R, kind="ExternalInput").ap(),
        "ones": nc.dram_tensor("ones", [1, R], F32R, kind="ExternalInput").ap(),
        "xb": nc.dram_tensor("xb", [N, D], BF16, kind="ExternalInput").ap(),
        "out": nc.dram_tensor("out", [R, D], F32, kind="ExternalOutput").ap(),
    }
    with tile.TileContext(nc) as tc:
        _emit(nc, tc, aps)
    nc.compile()
    return nc


_NC_CACHE = None
LAST_RESULTS = None


def _get_nc():
    global _NC_CACHE
    if _NC_CACHE is None:
        _NC_CACHE = build()
    return _NC_CACHE


def make_in_maps(x, Wq, bq, Wk):
    x = np.ascontiguousarray(np.asarray(x, dtype=np.float32))
    xT = np.ascontiguousarray(x.T)
    # xTb[j, p, e, n] = xT[e*128 + p, j*512 + n]: per-(j,p) contiguous 16KB
    # blocks so the phase-B stream DMAs at full descriptor size.
    xTb = np.ascontiguousarray(
        xT.reshape(EC, PT, JC, 512).transpose(2, 1, 0, 3)
    )
    wk64 = np.asarray(Wk, dtype=np.float64)
    mw = np.ascontiguousarray(
        (np.asarray(Wq, dtype=np.float64).T @ wk64).astype(np.float32)
    )
    cw = np.ascontiguousarray(
        (np.asarray(bq, dtype=np.float64) @ wk64).astype(np.float32).reshape(1, D)
    )
    ones = np.ones((1, R), dtype=np.float32)
    xb = x.astype(ml_dtypes.bfloat16)
    in_maps = []
    for c in range(NCORES):
        in_maps.append(
            {
                "xTb": xTb,
                "xTs": np.ascontiguousarray(xT[:, c * R : (c + 1) * R]),
                "mw": mw,
                "cw": cw,
                "ones": ones,
                "xb": xb,
            }
        )
    return in_maps


def kernel(x, Wq, bq, Wk, bk):
    # bk only shifts each score row by a constant, which softmax cancels.
    del bk
    in_maps = make_in_maps(x, Wq, bq, Wk)
    nc = _get_nc()
    kwargs = {}
    if os.environ.get("K_TRACE_DIR"):
        import tempfile

        kwargs["tmpdir"] = tempfile.mkdtemp(dir=os.environ["K_TRACE_DIR"])
    res = run_bass_kernel_spmd(nc, in_maps, core_ids=list(range(NCORES)), **kwargs)
    global LAST_RESULTS
    LAST_RESULTS = res
    return np.concatenate(
        [np.asarray(res.results[c]["out"], dtype=np.float32) for c in range(NCORES)],
        axis=0,
    )
